# revision 34
# baseline (speedup 1.0000x reference)
"""Trainium2 Bass kernel for nn_DiffusionClassifier (dense_mlp).

Data-parallel over batch across 8 NeuronCores (128 samples/core, params
replicated).  Per core:
  conv backbone (3x conv3x3+BN+ReLU+maxpool) as shifted matmuls,
  forward diffusion x_t, per-class 3-layer MLP.

Error phase uses the Gram trick:  with score = W3^T h2 (+b3) and
x0_pred - f = (s1/sa)(n - score),
  errors[c,b] = (s1/sa)^2 (||n||^2 - 2 h2.(W3 n) + h2.(G h2))
with G_c = W3_c W3_c^T precomputed on host.  This removes the 10x4096-wide
elementwise diff/square tail entirely.  The "u" matmuls u_c = -2 W3_c n are
independent of the conv/MLP chain, so they are emitted first and soak up PE
while their 21 MB of weights stream from HBM.

Pooling is restructured as tensor-tensor max pairs (DVE from PSUM, GpSimd
for the second level, ACT for relu+bias+bf16), conv1 bias rides a constant
ones-channel folded into the matmul.

Self-contained: hardcodes shapes; host-side numpy does only prep
(padding, weight fold/permute/cast, Gram matrices, time embedding).
"""

import sys

sys.path.insert(0, "/opt/trn_rl_repo")

import numpy as np
import ml_dtypes

import concourse.bass as bass
import concourse.tile as tile
import concourse.mybir as mybir
from concourse import bacc
from concourse.masks import make_identity
from concourse.bass_utils import run_bass_kernel_spmd

F32 = mybir.dt.float32
BF16 = mybir.dt.bfloat16
F8 = mybir.dt.float8e4
AL = mybir.AluOpType
AF = mybir.ActivationFunctionType
BF16_NP = ml_dtypes.bfloat16
FP8_NP = ml_dtypes.float8_e4m3fn
W8SCALE = 64.0

NCORES = 8
BC = 128  # samples per core
NCLS, HID, FEAT, TDIM = 10, 256, 4096, 128
BN_EPS = 1e-5

# device sample order: sdev = 64*g + slot ; global-in-core b = 8*(slot//4) + 4*g + slot%4
_PERM = np.array(
    [8 * (s // 4) + 4 * g + (s % 4) for g in (0, 1) for s in range(64)], dtype=np.int64
)

_BUILD_CACHE = {}


def _build(has_b3: bool):
    nc = bacc.Bacc("TRN2", target_bir_lowering=False, debug=False, enable_asserts=True)

    d = {}
    def din(name, shape, dt):
        d[name] = nc.dram_tensor(name, list(shape), dt, kind="ExternalInput").ap()

    din("xim", (16, 2, 28, 4, 32, 32), F8)
    din("npre", (128, 32, 128), BF16)
    din("timeT", (128, 128), BF16)
    din("sa_full", (128, 128), F32)
    din("negsc", (128, 1), F32)
    din("nsq", (128, NCLS if has_b3 else 1), F32)
    din("w1t", (128, 64), BF16)
    din("w2t", (9, 128, 128), BF16)
    din("b2e", (128, 1), F32)
    din("w3t", (9, 128, 256), BF16)
    din("b3e", (128, 2), F32)
    din("sW1p", (33, 128, NCLS, 256), F8)
    din("sW2p", (128, NCLS, 2, 256), BF16)
    din("w3u", (2, 32, 128, 5, 256), F8)
    din("Gp", (128, NCLS, 2, 256), BF16)
    din("noiseT", (128, 32, 128), BF16)
    din("sb1d", (128, 2 * NCLS), F32)
    din("sb2d", (128, 2 * NCLS), F32)
    if has_b3:
        din("vrow", (NCLS, 256), F32)
    out_d = nc.dram_tensor("out", [128, NCLS], F32, kind="ExternalOutput").ap()

    with tile.TileContext(nc) as tc:
        with tc.tile_pool(name="consts", bufs=1) as consts, tc.tile_pool(
            name="arena", bufs=1
        ) as arena:
            # ---- constants in SBUF (only conv1/u-phase inputs DMA'd now;
            # the rest are emitted after conv1 so they don't head-of-line
            # block the xim stream on the sync queue) ----
            w1t_s = consts.tile([128, 64], BF16)
            nc.sync.dma_start(w1t_s[:], d["w1t"])
            noiseT_s = consts.tile([128, 32, 128], BF16)
            nc.sync.dma_start(noiseT_s[:], d["noiseT"])
            w2t_s = consts.tile([128, 9, 128], BF16)
            b2e_s = consts.tile([128, 1], F32)
            w3t_s = consts.tile([128, 9, 256], BF16)
            b3e_s = consts.tile([128, 2], F32)
            sb1d_s = consts.tile([128, 2 * NCLS], F32)
            sb2d_s = consts.tile([128, 2 * NCLS], F32)
            sa_s = consts.tile([128, 128], F32)
            negsc_s = consts.tile([128, 1], F32)
            nsq_s = consts.tile([128, NCLS if has_b3 else 1], F32)
            sW2s = consts.tile([128, NCLS, 2, 256], BF16)
            Gp_s = consts.tile([128, NCLS, 2, 256], BF16)
            npre_s = consts.tile([128, 32, 128], BF16)
            identb = consts.tile([128, 128], BF16)
            make_identity(nc, identb[:])
            if has_b3:
                vrow_s = consts.tile([NCLS, 256], F32)

            def emit_late_consts():
                nc.sync.dma_start(w2t_s[:], d["w2t"].rearrange("s c o -> c s o"))
                nc.sync.dma_start(w3t_s[:], d["w3t"].rearrange("s c o -> c s o"))
                nc.sync.dma_start(b2e_s[:], d["b2e"])
                nc.sync.dma_start(b3e_s[:], d["b3e"])
                nc.sync.dma_start(sb1d_s[:], d["sb1d"])
                nc.sync.dma_start(sb2d_s[:], d["sb2d"])
                nc.sync.dma_start(sa_s[:], d["sa_full"])
                nc.sync.dma_start(negsc_s[:], d["negsc"])
                nc.sync.dma_start(nsq_s[:], d["nsq"])
                if has_b3:
                    nc.sync.dma_start(vrow_s[:], d["vrow"])

            # persistent arena buffers
            X2 = arena.tile([128, 64, 18, 18], BF16, tag="X2", name="X2")
            X3 = arena.tile([128, 128, 10, 10], BF16, tag="X3", name="X3")
            combT_bf = arena.tile([128, 33, 128], BF16, tag="combT", name="combT_bf")
            u_sb = arena.tile([128, NCLS, 256], BF16, tag="u_sb", name="u_sb")
            z1all = arena.tile([128, NCLS, 256], BF16, tag="z1all", name="z1all")
            logits = arena.tile([128, NCLS], F32, tag="logits", name="logits")
            err = arena.tile([128, NCLS], F32, tag="err", name="err")

            # PE warmup: ~5us of dense no-dep matmuls so the HAM clock-gate
            # opens (4/8 -> 8/8) before the real work arrives.
            with tc.tile_pool(name="warm", bufs=1) as warmp, tc.tile_pool(
                name="pswarm", bufs=1, space="PSUM"
            ) as pswp:
                wg = warmp.tile([128, 128], BF16)
                nc.gpsimd.memset(wg[:], 0.0)
                psw = pswp.tile([128, 128], F32)
                for _ in range(48):
                    nc.tensor.matmul(psw[:], wg[:], wg[:], start=True, stop=True)

            # zero only the padding borders of X2/X3
            nc.vector.memset(X2[:, :, 0, :], 0.0)
            nc.vector.memset(X2[:, :, 17, :], 0.0)
            nc.vector.memset(X2[:, :, 1:17, 0], 0.0)
            nc.vector.memset(X2[:, :, 1:17, 17], 0.0)

            # ============ u-phase: u_c = -2 W3_c n  (noise stationary) =========
            # Interleaved into conv1/conv2 emission (2 chunks per iteration):
            # every engine stream is FIFO, so emitting it as one block would
            # head-of-line-block the conv matmuls behind 21 MB of DMA.
            w3pool = tc.alloc_tile_pool(name="w3pool", bufs=8)
            psUp = tc.alloc_tile_pool(name="psU", bufs=1, space="PSUM")
            ucur = [0]
            ustate = {}

            def emit_u(n):
                for _ in range(n):
                    t = ucur[0]
                    if t >= 64:
                        return
                    g, ch = t // 32, t % 32
                    if ch == 0:
                        ustate[g] = [
                            psUp.tile(
                                [128, 2, 256], F32, tag=f"u{i}", name=f"psU_{g}_{i}"
                            )
                            for i in range(3)
                        ]
                    psU = ustate[g]
                    wchunk = w3pool.tile([128, 5, 256], F8, tag="w3c")
                    nc.gpsimd.dma_start(wchunk[:], d["w3u"][g, ch])
                    for i, cc in enumerate((0, 2, 4)):
                        w = 2 if cc < 4 else 1
                        nc.tensor.matmul(
                            psU[i][:, 0:w, :].rearrange("p a b -> p (a b)"),
                            noiseT_s[:, ch, :],
                            wchunk[:, cc : cc + w, :].rearrange("p a b -> p (a b)"),
                            start=(ch == 0),
                            stop=(ch == 31),
                            skip_group_check=True,
                        )
                    if ch == 31:
                        for cc in range(5):
                            nc.scalar.activation(
                                u_sb[:, 5 * g + cc, :],
                                psU[cc // 2][:, cc % 2, :],
                                AF.Identity,
                                scale=1.0 / W8SCALE,
                            )
                    ucur[0] += 1

            # ================= stage 1: conv 3->64, pool -> X2 =================
            with tc.tile_pool(name="io27", bufs=3) as io27, tc.tile_pool(
                name="ev1", bufs=3
            ) as ev1, tc.tile_pool(name="ps1", bufs=2, space="PSUM") as ps1:
                for bc in range(16):
                    X27 = io27.tile([128, 4, 32, 32], F8, tag="X27")
                    for g in (0, 1):
                        nc.sync.dma_start(
                            X27[32 * g : 32 * g + 28, :, :, :],
                            d["xim"][bc, g],
                        )
                    for bq in range(4):
                        ps = ps1.tile([128, 2, 16, 32], F32, tag="s1ps")
                        for ih in (0, 1):
                            nc.tensor.matmul(
                                ps[0:64, ih],
                                w1t_s[0:28, :],
                                X27[0:28, bq, 16 * ih : 16 * ih + 16, :],
                                start=True,
                                stop=True,
                            )
                            nc.tensor.matmul(
                                ps[64:128, ih],
                                w1t_s[32:60, :],
                                X27[32:60, bq, 16 * ih : 16 * ih + 16, :],
                                tile_position=(32, 64),
                                start=True,
                                stop=True,
                            )
                        slot = 4 * bc + bq
                        dst = X2[:, slot, 1:17, 1:17]  # [p, 16, 16] = (ih,i) x j
                        # one DVE 2x2 reduce from PSUM; relu on ACT (bias is
                        # already in psum via the ones-channel)
                        tD = ev1.tile([128, 16, 16], F32, tag="s1d")
                        nc.vector.reduce_max(
                            tD[:],
                            ps[:]
                            .rearrange("p s i j -> p (s i) j")
                            .rearrange("p (c a) (j b) -> p c j a b", a=2, b=2),
                            axis=mybir.AxisListType.XY,
                        )
                        nc.scalar.activation(dst, tD[:], AF.Relu)
                        if bq == 3:
                            emit_u(2)

            nc.vector.memset(X3[:, :, 0, :], 0.0)
            nc.vector.memset(X3[:, :, 9, :], 0.0)
            nc.vector.memset(X3[:, :, 1:9, 0], 0.0)
            nc.vector.memset(X3[:, :, 1:9, 9], 0.0)
            emit_late_consts()
            nc.sync.dma_start(combT_bf[:, 32, :], d["timeT"])

            # ================= stage 2: conv 64->128, pool -> X3 ================
            with tc.tile_pool(name="ev2", bufs=3) as ev2, tc.tile_pool(
                name="ps2", bufs=4, space="PSUM"
            ) as ps2:
                for w in range(0, 32, 2):
                    emit_u(2)
                    ks = [w, w + 1]
                    pg = {}
                    for g in (0, 1):
                        for k in ks:
                            pg[(g, k)] = ps2.tile(
                                [128, 2, 16, 16], F32, tag="s2ps", name=f"s2ps_{g}_{k}"
                            )
                    for s in range(9):
                        di, dj = divmod(s, 3)
                        for g in (0, 1):
                            lhsT = w2t_s[64 * g : 64 * g + 64, s, :]
                            for k in ks:
                                nc.tensor.matmul(
                                    pg[(g, k)][:],
                                    lhsT,
                                    X2[
                                        64 * g : 64 * g + 64,
                                        2 * k : 2 * k + 2,
                                        di : di + 16,
                                        dj : dj + 16,
                                    ],
                                    start=(s == 0),
                                    stop=(s == 8),
                                )
                    for g in (0, 1):
                        for k in ks:
                            p = pg[(g, k)]
                            tD = ev2.tile([128, 16, 8], F32, tag="s2d")
                            nc.vector.reduce_max(
                                tD[:],
                                p[:]
                                .rearrange("p s i j -> p (s i) j")
                                .rearrange("p (c a) (j b) -> p c j a b", a=2, b=2),
                                axis=mybir.AxisListType.XY,
                            )
                            sd = 64 * g + 2 * k
                            nc.scalar.activation(
                                X3[:, sd : sd + 2, 1:9, 1:9],
                                tD[:].rearrange("p (s i) j -> p s i j", s=2),
                                AF.Relu,
                                bias=b2e_s[:, 0:1],
                            )

            emit_u(64)  # drain any leftover u-chunks
            psUp.release()
            w3pool.release()

            nc.sync.dma_start(npre_s[:], d["npre"])

            # ============ stage 3: conv 128->256, pool -> combT_bf (features) ===
            with tc.tile_pool(name="ev3", bufs=3) as ev3, tc.tile_pool(
                name="ps3", bufs=3, space="PSUM"
            ) as ps3:
                for oh in (0, 1):
                    for w in range(0, 16, 2):
                        ks = [w, w + 1]
                        pk = {
                            k: ps3.tile([128, 8, 8, 8], F32, tag="s3ps", name=f"s3ps_{k}")
                            for k in ks
                        }
                        for s in range(9):
                            di, dj = divmod(s, 3)
                            lhsT = w3t_s[:, s, 128 * oh : 128 * oh + 128]
                            for k in ks:
                                nc.tensor.matmul(
                                    pk[k][:],
                                    lhsT,
                                    X3[:, 8 * k : 8 * k + 8, di : di + 8, dj : dj + 8],
                                    start=(s == 0),
                                    stop=(s == 8),
                                )
                        for k in ks:
                            p = pk[k]
                            tD = ev3.tile([128, 32, 4], F32, tag="s3d")
                            nc.vector.reduce_max(
                                tD[:],
                                p[:]
                                .rearrange("p s i j -> p (s i) j")
                                .rearrange("p (c a) (j b) -> p c j a b", a=2, b=2),
                                axis=mybir.AxisListType.XY,
                            )
                            dest = combT_bf[
                                :, 16 * oh : 16 * oh + 16, 8 * k : 8 * k + 8
                            ].rearrange("p (i j) b -> p b i j", i=4, j=4)
                            nc.scalar.activation(
                                dest,
                                tD[:].rearrange("p (b i) j -> p b i j", b=8),
                                AF.Relu,
                                bias=b3e_s[:, oh : oh + 1],
                            )
                    # x_t for this half: comb = sa*f + s1*noise (in-place)
                    sl_ = slice(16 * oh, 16 * oh + 16)
                    sab = sa_s[:].unsqueeze(1).to_broadcast((128, 16, 128))
                    nc.vector.tensor_mul(combT_bf[:, sl_, :], combT_bf[:, sl_, :], sab)
                    nc.vector.tensor_add(
                        combT_bf[:, sl_, :], combT_bf[:, sl_, :], npre_s[:, sl_, :]
                    )

            # ================= MLP layer 1 (all classes) ========================
            with tc.tile_pool(name="w1pool", bufs=8) as w1pool, tc.tile_pool(
                name="psL1", bufs=1, space="PSUM"
            ) as psL1:
                pp = [
                    psL1.tile([128, 512], F32, tag=f"l1ps{i}", name=f"l1ps_{i}")
                    for i in range(5)
                ]
                for fc in [32] + list(range(32)):
                    wt = w1pool.tile([128, NCLS, 256], F8, tag="w1s")
                    nc.gpsimd.dma_start(wt[:], d["sW1p"][fc])
                    for ci in range(5):
                        nc.tensor.matmul(
                            pp[ci][:],
                            combT_bf[:, fc, :],
                            wt[:, 2 * ci : 2 * ci + 2, :].rearrange(
                                "p a b -> p (a b)"
                            ),
                            start=(fc == 32),
                            stop=(fc == 31),
                            skip_group_check=True,
                        )
                nc.gpsimd.dma_start(sW2s[:], d["sW2p"])
                nc.gpsimd.dma_start(Gp_s[:], d["Gp"])
                for c in range(NCLS):
                    nc.vector.tensor_copy(
                        z1all[:, c, :], pp[c // 2][:, 256 * (c % 2) : 256 * (c % 2) + 256]
                    )

            # ========== per-class L2, z = h2 G, dot products ====================
            with tc.tile_pool(name="mlp", bufs=3) as mlp, tc.tile_pool(
                name="scr", bufs=3
            ) as scrp, tc.tile_pool(name="psT", bufs=2, space="PSUM") as psT, tc.tile_pool(
                name="psZ", bufs=2, space="PSUM"
            ) as psZp:
                for c in range(NCLS):
                    tp = psT.tile([128, 2, 128], BF16, tag="tp")
                    for hc in (0, 1):
                        nc.tensor.transpose(
                            tp[:, hc, :], z1all[:, c, 128 * hc : 128 * hc + 128], identb[:]
                        )
                    h1T = mlp.tile([128, 2, 128], BF16, tag="h1T")
                    for hc in (0, 1):
                        nc.scalar.activation(
                            h1T[:, hc, :],
                            tp[:, hc, :],
                            AF.Relu,
                            bias=sb1d_s[:, 2 * c + hc : 2 * c + hc + 1],
                            scale=1.0 / W8SCALE,
                        )
                    z2 = psT.tile([128, 2, 128], F32, tag="z2")
                    for hp in (0, 1):
                        for hc in (0, 1):
                            nc.tensor.matmul(
                                z2[:, hp, :],
                                sW2s[:, c, hc, 128 * hp : 128 * hp + 128],
                                h1T[:, hc, :],
                                start=(hc == 0 and hp == 0),
                                stop=(hc == 1),
                                skip_group_check=True,
                            )
                    h2T = mlp.tile([128, 2, 128], BF16, tag="h2T")
                    for hc in (0, 1):
                        nc.scalar.activation(
                            h2T[:, hc, :],
                            z2[:, hc, :],
                            AF.Relu,
                            bias=sb2d_s[:, 2 * c + hc : 2 * c + hc + 1],
                        )
                    tp2 = psT.tile([128, 2, 128], BF16, tag="tp2")
                    for hc in (0, 1):
                        nc.tensor.transpose(tp2[:, hc, :], h2T[:, hc, :], identb[:])
                    h2b = mlp.tile([128, 256], BF16, tag="h2b")
                    nc.vector.tensor_copy(h2b[:], tp2[:].rearrange("p s b -> p (s b)"))
                    psZ = psZp.tile([128, 256], F32, tag="psZ")
                    for hc in (0, 1):
                        nc.tensor.matmul(
                            psZ[:],
                            h2T[:, hc, :],
                            Gp_s[:, c, hc, :],
                            start=(hc == 0),
                            stop=(hc == 1),
                            skip_group_check=True,
                        )
                    nrow = 3 if has_b3 else 2
                    prod = scrp.tile([128, nrow, 256], F32, tag="prod")
                    nc.vector.tensor_mul(prod[:, 0, :], h2b[:], u_sb[:, c, :])
                    nc.vector.tensor_mul(prod[:, 1, :], h2b[:], psZ[:])
                    if has_b3:
                        nc.vector.tensor_mul(
                            prod[:, 2, :],
                            h2b[:],
                            vrow_s[c : c + 1, :].to_broadcast((128, 256)),
                        )
                    nc.vector.reduce_sum(
                        err[:, c : c + 1],
                        prod[:].rearrange("p r h -> p (r h)").unsqueeze(1),
                        axis=mybir.AxisListType.X,
                    )
                if has_b3:
                    nc.vector.tensor_add(err[:], err[:], nsq_s[:])
                    nc.vector.tensor_scalar_mul(logits[:], err[:], negsc_s[:, 0:1])
                else:
                    nc.vector.tensor_scalar(
                        logits[:], err[:], nsq_s[:, 0:1], negsc_s[:, 0:1],
                        AL.add, AL.mult,
                    )
                nc.sync.dma_start(out_d, logits[:])

    nc.compile()
    return nc


def _host_prep(inputs):
    x = np.asarray(inputs["x"], np.float32)
    noise = np.asarray(inputs["noise"], np.float32)
    t = np.asarray(inputs["t"])
    B = x.shape[0]

    betas = np.linspace(0.0001, 0.02, 10, dtype=np.float32)
    ac = np.cumprod((1.0 - betas).astype(np.float32)).astype(np.float32)
    a_t = ac[t]
    sa = np.sqrt(a_t).astype(np.float32)
    s1 = np.sqrt(1.0 - a_t).astype(np.float32)

    half = TDIM // 2
    freqs = np.exp(
        np.arange(half, dtype=np.float32) * (-np.log(10000.0) / (half - 1))
    ).astype(np.float32)
    ang = t.astype(np.float32)[:, None] * freqs[None, :]
    t_emb = np.concatenate([np.sin(ang), np.cos(ang)], axis=1).astype(np.float32)

    xpad = np.zeros((B, 3, 34, 34), np.float32)
    xpad[:, :, 1:33, 1:33] = x
    win = np.lib.stride_tricks.sliding_window_view(xpad, (32, 32), axis=(2, 3))
    # win[b, c, di, dj, i, j] -> xim_all[b, (3*di+dj)*3+c, i, j]; chan 27 = ones
    xim_all = np.ones((B, 28, 32, 32), np.float32)
    xim_all[:, 0:27] = win.transpose(0, 2, 3, 1, 4, 5).reshape(B, 27, 32, 32)

    def bnfold(i):
        g, be, m, v, b = (
            np.asarray(inputs[f"g{i}"], np.float32),
            np.asarray(inputs[f"be{i}"], np.float32),
            np.asarray(inputs[f"m{i}"], np.float32),
            np.asarray(inputs[f"v{i}"], np.float32),
            np.asarray(inputs[f"b{i}"], np.float32),
        )
        sc = g / np.sqrt(v + BN_EPS)
        return sc, ((b - m) * sc + be).astype(np.float32)

    sc1, bf1 = bnfold(1)
    sc2, bf2 = bnfold(2)
    sc3, bf3 = bnfold(3)
    w1 = np.asarray(inputs["w1"], np.float32) * sc1[:, None, None, None]
    w2 = np.asarray(inputs["w2"], np.float32) * sc2[:, None, None, None]
    w3 = np.asarray(inputs["w3"], np.float32) * sc3[:, None, None, None]

    w1t = np.zeros((128, 64), np.float32)
    for q in (0, 1):
        for s in range(9):
            di, dj = divmod(s, 3)
            for cch in range(3):
                w1t[32 * q + 3 * s + cch, :] = w1[:, cch, di, dj]
        w1t[32 * q + 27, :] = bf1  # bias via ones-channel

    w2t = np.zeros((9, 128, 128), np.float32)
    for s in range(9):
        di, dj = divmod(s, 3)
        w2t[s, 0:64, :] = w2[:, :, di, dj].T
        w2t[s, 64:128, :] = w2[:, :, di, dj].T
    b2e = bf2.astype(np.float32)[:, None]

    w3t = np.zeros((9, 128, 256), np.float32)
    for s in range(9):
        di, dj = divmod(s, 3)
        w3t[s] = w3[:, :, di, dj].T
    b3e = bf3.reshape(2, 128).T.copy().astype(np.float32)

    # f-permutation for device comb layout: chunk j (0..31), partition p:
    #   f = ((j//16)*128 + p)*16 + (j%16)
    jj, pp_ = np.meshgrid(np.arange(32), np.arange(128), indexing="ij")
    fidx = ((jj // 16) * 128 + pp_) * 16 + (jj % 16)  # [32, 128]
    fflat = fidx.reshape(-1)

    sW1 = np.asarray(inputs["sW1"], np.float32)  # [10, 4224, 256]
    sW1p = np.zeros((33, 128, NCLS, 256), np.float32)
    # [10, 32*128, 256] -> [32, 128, 10, 256]
    sW1p[0:32] = sW1[:, fflat, :].reshape(NCLS, 32, 128, 256).transpose(1, 2, 0, 3)
    sW1p[32] = sW1[:, FEAT : FEAT + 128, :].transpose(1, 0, 2)

    sW2 = np.asarray(inputs["sW2"], np.float32)  # [10, 256, 256]
    sW2p = sW2.reshape(NCLS, 2, 128, 256).transpose(2, 0, 1, 3)  # [128,10,2,256]

    sW3 = np.asarray(inputs["sW3"], np.float32)  # [10, 256, 4096]
    # w3u[g, ch, p, cc, h] = -2*sW3[5g+cc, h, 128ch+p]
    w3u = (
        (-2.0 * sW3)
        .transpose(0, 2, 1)  # [10, 4096, 256]
        .reshape(2, 5, 32, 128, 256)
        .transpose(0, 2, 3, 1, 4)  # [2, 32, 128, 5, 256]
    )
    w3u = np.ascontiguousarray(w3u, np.float32)

    # Gram matrices  G_c = W3_c W3_c^T  [10, 256, 256]
    G = np.einsum("chf,ckf->chk", sW3, sW3, optimize=True).astype(np.float32)
    Gp = G.reshape(NCLS, 2, 128, 256).transpose(2, 0, 1, 3)  # [128,10,2,256]

    sb1 = np.asarray(inputs["sb1"], np.float32)
    sb2 = np.asarray(inputs["sb2"], np.float32)
    sb3 = np.asarray(inputs["sb3"], np.float32)
    sb1d = np.zeros((128, 2 * NCLS), np.float32)
    sb2d = np.zeros((128, 2 * NCLS), np.float32)
    for c in range(NCLS):
        for hc in (0, 1):
            sb1d[:, 2 * c + hc] = sb1[c, 128 * hc : 128 * hc + 128]
            sb2d[:, 2 * c + hc] = sb2[c, 128 * hc : 128 * hc + 128]
    has_b3 = bool(np.any(sb3))

    common = dict(
        w1t=w1t.astype(BF16_NP),
        w2t=w2t.astype(BF16_NP),
        b2e=b2e,
        w3t=w3t.astype(BF16_NP),
        b3e=b3e,
        sW1p=(np.ascontiguousarray(sW1p) * W8SCALE).astype(FP8_NP),
        sW2p=np.ascontiguousarray(sW2p).astype(BF16_NP),
        w3u=(w3u * W8SCALE).astype(FP8_NP),
        Gp=np.ascontiguousarray(Gp).astype(BF16_NP),
        sb1d=sb1d,
        sb2d=sb2d,
    )
    if has_b3:
        # score = W3^T h2 + b3:  ||n - b3 - W3^T h2||^2
        #  = ||n-b3||^2 - 2 h2.(W3 (n-b3)) + h2.(G h2)
        #  = nsqc  + h2.u + h2.(2 W3 b3) + h2.(G h2)   (u = -2 W3 n)
        vrow = 2.0 * np.einsum("chf,cf->ch", sW3, sb3).astype(np.float32)
        common["vrow"] = vrow

    in_maps = []
    for k in range(NCORES):
        b0 = k * BC
        pg = b0 + _PERM
        nslice = noise[pg]  # [sdev, f]
        npre_full = (s1[pg][:, None] * nslice)[:, fflat]  # [sdev, (j,p)]
        npre = npre_full.reshape(128, 32, 128).transpose(2, 1, 0)  # [p, j, sdev]
        m = dict(common)
        m["xim"] = np.ascontiguousarray(
            xim_all[b0 : b0 + BC]
            .reshape(16, 2, 4, 28, 32, 32)
            .transpose(0, 1, 3, 2, 4, 5)
        ).astype(FP8_NP)
        m["npre"] = np.ascontiguousarray(npre).astype(BF16_NP)
        m["noiseT"] = np.ascontiguousarray(
            nslice.reshape(128, 32, 128).transpose(2, 1, 0)
        ).astype(BF16_NP)
        m["timeT"] = np.ascontiguousarray(t_emb[pg].T).astype(BF16_NP)
        m["sa_full"] = np.ascontiguousarray(
            np.tile(sa[pg][None, :], (128, 1)), np.float32
        )
        m["negsc"] = (-((s1[pg] / sa[pg]) ** 2)).astype(np.float32)[:, None]
        nsq = (nslice**2).sum(axis=1).astype(np.float32)
        if has_b3:
            nsqc = (
                nsq[:, None]
                - 2.0 * nslice @ sb3.T
                + (sb3**2).sum(axis=1)[None, :]
            ).astype(np.float32)
            m["nsq"] = nsqc
        else:
            m["nsq"] = nsq[:, None]
        in_maps.append(m)
    return in_maps, has_b3


def kernel(**inputs):
    in_maps, has_b3 = _host_prep(inputs)
    if has_b3 not in _BUILD_CACHE:
        _BUILD_CACHE[has_b3] = _build(has_b3)
    nc = _BUILD_CACHE[has_b3]
    res = run_bass_kernel_spmd(nc, in_maps, core_ids=list(range(NCORES)))
    out = np.zeros((NCORES * BC, NCLS), np.float32)
    for k in range(NCORES):
        out[k * BC + _PERM] = res.results[k]["out"]
    return out
